# revision 1
# baseline (speedup 1.0000x reference)
"""Multi-head self-attention (B=4, S=2048, D=1024, H=16, RoPE, causal) on 8 trn2 cores.

Sharding: core c -> batch c//2, heads [8*(c%2), 8*(c%2)+8)   (2 cores per batch,
each doing 8 of the 16 heads).  Each core computes its partial output
projection out^T [1024, 2048]; host sums the two halves per batch and
transposes back.

All matmuls run as float32r (TF32-like, full PE rate).  Layout is transposed
throughout: x^T [D,S] in SBUF, Q^T/K^T [dk,s], scores^T [k,q] (softmax sum
via a ones-column appended to V in the attn@V matmul), out^T [o,s].
"""
import sys
sys.path.insert(0, "/opt/trn_rl_repo")
import math
from contextlib import ExitStack
import numpy as np
import ml_dtypes

import concourse.bass as bass
import concourse.bacc as bacc
import concourse.mybir as mybir
from concourse.tile import TileContext
from concourse.bass_utils import run_bass_kernel_spmd

F32 = mybir.dt.float32
F32R = mybir.dt.float32r
BF16 = mybir.dt.bfloat16
ATTN_BF16 = False
ADT = BF16 if ATTN_BF16 else F32R

B, S, D, H, DK = 4, 2048, 1024, 16, 64
NCORES = 8
NPAIR = 4               # head pairs per core
QC = 512                # q chunk (matmul moving free size)
NQC = S // QC           # 4
KC = 128                # k chunk (scores psum partition dim)
NKC = S // KC           # 16
SC = 512                # s chunk for projections / outproj
NSC = S // SC           # 4

_BUILT = {}


def _build_nc():
    nc = bacc.Bacc()

    xT_d = [nc.declare_dram_parameter(f"xT{sc}", [D, SC], F32R, isOutput=False) for sc in range(NSC)]
    wq_d = nc.declare_dram_parameter("wqT", [D, 512], F32R, isOutput=False)
    wk_d = nc.declare_dram_parameter("wkT", [D, 512], F32R, isOutput=False)
    wv_d = nc.declare_dram_parameter("wvT", [D, 512], F32R, isOutput=False)
    wo_d = nc.declare_dram_parameter("woT", [512, D], F32R, isOutput=False)
    ctab_d = nc.declare_dram_parameter("ctab", [128, S], F32, isOutput=False)
    stab_d = nc.declare_dram_parameter("stab", [128, S], F32, isOutput=False)
    mk_d = nc.declare_dram_parameter("masks", [KC, 4, QC], ADT, isOutput=False)
    id_d = nc.declare_dram_parameter("ident", [128, 128], ADT, isOutput=False)
    idT_d = nc.declare_dram_parameter("identT", [128, 64], F32, isOutput=False)
    ones16_d = nc.declare_dram_parameter("ones16", [128, NKC], ADT, isOutput=False)
    out_d = nc.declare_dram_parameter("outP", [D, S], F32, isOutput=True)

    swapmask = [i ^ 1 for i in range(32)]

    with TileContext(nc) as tc, ExitStack() as ctx:
        ep = ctx.enter_context
        consts = ep(tc.tile_pool(name="consts", bufs=1))
        xpool = ep(tc.tile_pool(name="xt", bufs=12))
        wpool = ep(tc.tile_pool(name="w", bufs=36))
        wopool = ep(tc.tile_pool(name="wo", bufs=8))
        vpool = ep(tc.tile_pool(name="vraw", bufs=1))
        rupool = ep(tc.tile_pool(name="ru", bufs=2))
        rvpool = ep(tc.tile_pool(name="rv", bufs=2))
        qkpool = ep(tc.tile_pool(name="qk", bufs=4))
        v1pool = ep(tc.tile_pool(name="v1", bufs=3))
        ppool = ep(tc.tile_pool(name="pT", bufs=2))
        orawpool = ep(tc.tile_pool(name="oraw", bufs=1))
        sumpool = ep(tc.tile_pool(name="sums", bufs=2))
        recpool = ep(tc.tile_pool(name="rec", bufs=2))
        otpool = ep(tc.tile_pool(name="oT", bufs=4))
        obpool = ep(tc.tile_pool(name="ob", bufs=2))
        drpool = ep(tc.tile_pool(name="dr", bufs=4, space="DRAM"))
        psA = ep(tc.tile_pool(name="psA", bufs=2, space="PSUM"))
        psB = ep(tc.tile_pool(name="psB", bufs=2, space="PSUM"))
        psO = ep(tc.tile_pool(name="psO", bufs=2, space="PSUM"))

        ctab = consts.tile([128, S], F32)
        stab = consts.tile([128, S], F32)
        masks = consts.tile([KC, 4, QC], ADT)
        ident = consts.tile([128, 128], ADT)
        identT = consts.tile([128, 64], F32)
        nc.sync.dma_start(out=ctab, in_=ctab_d[:, :])
        nc.sync.dma_start(out=stab, in_=stab_d[:, :])
        nc.sync.dma_start(out=masks, in_=mk_d[:, :, :])
        nc.sync.dma_start(out=ident, in_=id_d[:, :])
        nc.sync.dma_start(out=identT, in_=idT_d[:, :])

        oTs = []
        for hp in range(NPAIR):
            # ---------------- projections for head pair hp ----------------
            wts = {}
            for pj, wsrc in enumerate((wq_d, wk_d, wv_d)):
                for ic in range(8):
                    w = wpool.tile([128, 128], F32R, tag="w")
                    nc.sync.dma_start(
                        out=w, in_=wsrc[ic * 128:(ic + 1) * 128,
                                        hp * 128:(hp + 1) * 128])
                    wts[(pj, ic)] = w

            qT = qkpool.tile([128, S], ADT, tag="qk")
            kT = qkpool.tile([128, S], ADT, tag="qk")
            vraw = vpool.tile([128, S], F32, tag="vraw")

            _sc = nc.named_scope(f"proj{hp}"); _sc.__enter__()
            for sc in range(NSC):
                xts = []
                for ic in range(8):
                    xt = xpool.tile([128, SC], F32R, tag="xt")
                    nc.sync.dma_start(
                        out=xt, in_=xT_d[sc][ic * 128:(ic + 1) * 128, :])
                    xts.append(xt)
                ssl = slice(sc * SC, (sc + 1) * SC)
                for pj in range(3):
                    ps = psA.tile([128, SC], F32, tag="psA")
                    for ic in range(8):
                        nc.tensor.matmul(ps, wts[(pj, ic)], xts[ic],
                                         start=(ic == 0), stop=(ic == 7))
                    if pj < 2:  # Q or K: RoPE directly from PSUM
                        dst = qT if pj == 0 else kT
                        sh = rupool.tile([128, SC], F32, tag="ru")
                        nc.vector.stream_shuffle(out=sh, in_=ps, mask=swapmask)
                        t1 = rvpool.tile([128, SC], F32, tag="rv")
                        nc.vector.tensor_mul(out=t1, in0=ps, in1=ctab[:, ssl])
                        t2 = rupool.tile([128, SC], F32, tag="ru")
                        nc.vector.tensor_mul(out=t2, in0=sh, in1=stab[:, ssl])
                        nc.vector.tensor_add(out=dst[:, ssl], in0=t1, in1=t2)
                    else:       # V: drain to SBUF for PE transpose
                        nc.scalar.copy(out=vraw[:, ssl], in_=ps)

            _sc.__exit__(None, None, None)
            # ---------------- V transpose: [dk, s] -> [s, dk] + ones col ----
            _sc = nc.named_scope(f"vt{hp}"); _sc.__enter__()
            v1s = []
            for h in range(2):
                v1 = v1pool.tile([128, NKC, 65], ADT, tag="v1")
                for half in range(2):
                    pvt = psB.tile([128, 512], F32, tag="big")
                    for j in range(8):
                        kc = half * 8 + j
                        nc.tensor.transpose(
                            pvt[:, j * 64:(j + 1) * 64],
                            vraw[h * 64:(h + 1) * 64, kc * 128:(kc + 1) * 128],
                            identT[h * 64:(h + 1) * 64, 0:64])
                    nc.vector.tensor_copy(
                        out=v1[:, half * 8:(half + 1) * 8, 0:64],
                        in_=pvt.rearrange("p (kc d) -> p kc d", d=64))
                nc.sync.dma_start(out=v1[:, :, 64:65],
                                  in_=ones16_d[:, :].unsqueeze(2))
                v1s.append(v1)

            _sc.__exit__(None, None, None)
            # ---------------- attention per head ----------------
            _sc = nc.named_scope(f"attn{hp}"); _sc.__enter__()
            oT = otpool.tile([128, S], F32R, tag="oT")
            oraw = orawpool.tile([128, S], F32, tag="oraw")
            for h in range(2):
                hs = slice(h * 64, (h + 1) * 64)
                sums = sumpool.tile([128, QC], F32, tag="sums")
                nc.vector.memset(sums, 1.0)
                for qc in range(NQC):
                    nact = 4 * qc + 4
                    qsl = slice(qc * QC, (qc + 1) * QC)
                    pquads = []
                    for pr in range(nact // 2):
                        psq = psB.tile([128, 1024], F32, tag="big")
                        for sl in range(2):
                            kc = pr * 2 + sl
                            csl = slice(sl * QC, (sl + 1) * QC)
                            moff = kc - 4 * qc
                            partial = moff >= 0
                            nc.tensor.matmul(
                                psq[:, csl],
                                kT[hs, kc * KC:(kc + 1) * KC],
                                qT[hs, qsl],
                                start=True, stop=(not partial))
                            if partial:
                                nc.tensor.matmul(
                                    psq[:, csl], ident, masks[:, moff, :],
                                    start=False, stop=True)
                        pq = ppool.tile([128, 1024], ADT, tag="pT")
                        nc.scalar.activation(
                            out=pq, in_=psq,
                            func=mybir.ActivationFunctionType.Exp, scale=0.125)
                        pquads.append(pq)
                    pso = psO.tile([65, QC], F32, tag="psO")
                    for kc in range(nact):
                        pr, sl = divmod(kc, 2)
                        nc.tensor.matmul(
                            pso, v1s[h][:, kc, :],
                            pquads[pr][:, sl * QC:(sl + 1) * QC],
                            start=(kc == 0), stop=(kc == nact - 1))
                    nc.vector.tensor_copy(out=oraw[hs, qsl], in_=pso[0:64, :])
                    nc.vector.tensor_copy(out=sums[32 * qc:32 * qc + 1, :],
                                          in_=pso[64:65, :])
                # batched normalization for this head
                rec = recpool.tile([128, QC], F32, tag="rec")
                nc.vector.reciprocal(out=rec, in_=sums)
                drt = drpool.tile([NQC, QC], F32)
                for qc in range(NQC):
                    nc.sync.dma_start(out=drt[qc:qc + 1, :],
                                      in_=rec[32 * qc:32 * qc + 1, :])
                for qc in range(NQC):
                    qsl = slice(qc * QC, (qc + 1) * QC)
                    recB = recpool.tile([128, QC], F32, tag="recB")
                    nc.sync.dma_start(out=recB[hs, :],
                                      in_=drt[qc:qc + 1, :].to_broadcast((64, QC)))
                    nc.vector.tensor_mul(out=oT[hs, qsl], in0=oraw[hs, qsl],
                                         in1=recB[hs, :])
            _sc.__exit__(None, None, None)
            oTs.append(oT)

        # ---------------- output projection ----------------
        _sc = nc.named_scope("outproj"); _sc.__enter__()
        for oc in range(8):
            wos = []
            for hp in range(NPAIR):
                w = wopool.tile([128, 128], F32R, tag="wo")
                nc.sync.dma_start(
                    out=w, in_=wo_d[hp * 128:(hp + 1) * 128,
                                    oc * 128:(oc + 1) * 128])
                wos.append(w)
            for sc in range(NSC):
                ps = psA.tile([128, SC], F32, tag="psA")
                for hp in range(NPAIR):
                    nc.tensor.matmul(ps, wos[hp],
                                     oTs[hp][:, sc * SC:(sc + 1) * SC],
                                     start=(hp == 0), stop=(hp == NPAIR - 1))
                ob = obpool.tile([128, SC], F32, tag="ob")
                nc.vector.tensor_copy(out=ob, in_=ps)
                nc.sync.dma_start(
                    out=out_d[oc * 128:(oc + 1) * 128, sc * SC:(sc + 1) * SC],
                    in_=ob)
        _sc.__exit__(None, None, None)

    nc.compile()
    return nc


def get_nc():
    if "nc" not in _BUILT:
        _BUILT["nc"] = _build_nc()
    return _BUILT["nc"]


def _host_prep(x, Wq, Wk, Wv, Wo, token_positions):
    pos = np.asarray(token_positions).astype(np.float32)
    half = DK // 2
    inv_freq = 1.0 / (10000.0 ** (np.arange(half, dtype=np.float32) * 2.0 / DK))
    ang = pos[:, None] * inv_freq[None, :]          # [S, 32]
    cos = np.cos(ang).astype(np.float32)            # [S, 32]
    sin = np.sin(ang).astype(np.float32)
    p = np.arange(128)
    j = (p % 64) // 2
    sign = np.where(p % 2 == 0, -1.0, 1.0).astype(np.float32)
    ctab = np.ascontiguousarray(cos[:, j].T)                      # [128, S]
    stab = np.ascontiguousarray(sin[:, j].T * sign[:, None])      # [128, S]

    kk = np.arange(KC)[:, None]
    qq = np.arange(QC)[None, :]
    adt = ml_dtypes.bfloat16 if ATTN_BF16 else np.float32
    masks = np.stack([np.where(qq >= kk + 128 * m, 0.0, -1e9)
                      for m in range(4)], axis=1).astype(adt)  # [KC,4,QC]
    ident = np.eye(128, dtype=adt)
    identT = np.vstack([np.eye(64, dtype=np.float32)] * 2)
    ones16 = np.ones((128, NKC), dtype=adt)

    in_maps = []
    for c in range(NCORES):
        b, hf = divmod(c, 2)
        xT = np.ascontiguousarray(x[b].T)           # [D, S]
        m = {}
        for sc in range(NSC):
            m[f"xT{sc}"] = np.ascontiguousarray(xT[:, sc * SC:(sc + 1) * SC])
        m["wqT"] = np.ascontiguousarray(Wq[hf * 512:(hf + 1) * 512, :].T)
        m["wkT"] = np.ascontiguousarray(Wk[hf * 512:(hf + 1) * 512, :].T)
        m["wvT"] = np.ascontiguousarray(Wv[hf * 512:(hf + 1) * 512, :].T)
        m["woT"] = np.ascontiguousarray(Wo[:, hf * 512:(hf + 1) * 512].T)
        m["ctab"] = ctab
        m["stab"] = stab
        m["masks"] = masks
        m["ident"] = ident
        m["identT"] = identT
        m["ones16"] = ones16
        in_maps.append(m)
    return in_maps


def run(inputs, trace=False, **kw):
    in_maps = _host_prep(**{k: np.asarray(v) for k, v in inputs.items()})
    nc = get_nc()
    res = run_bass_kernel_spmd(nc, in_maps, list(range(NCORES)), trace=trace, **kw)
    outs = [res.results[c]["outP"] for c in range(NCORES)]
    out = np.stack([(outs[2 * b] + outs[2 * b + 1]).T for b in range(B)])
    return out.astype(np.float32), res


def kernel(**inputs):
    out, _ = run(inputs, trace=False)
    return out



# revision 8
# speedup vs baseline: 1.2395x; 1.2395x over previous
"""Multi-head self-attention (B=4, S=2048, D=1024, H=16, RoPE, causal) on 8 trn2 cores.

Sharding: core c -> batch c//2, heads [8*(c%2), 8*(c%2)+8)   (2 cores per batch,
each doing 8 of the 16 heads).  Each core computes its partial output
projection out^T [1024, 2048]; host sums the two halves per batch and
transposes back.

All matmuls run as float32r (TF32-like, full PE rate).  Layout is transposed
throughout: x^T [D,S] in SBUF, Q^T/K^T [dk,s], scores^T [k,q] (softmax sum
via a ones-column appended to V in the attn@V matmul), out^T [o,s].
"""
import sys
sys.path.insert(0, "/opt/trn_rl_repo")
import math
from contextlib import ExitStack
import numpy as np
import ml_dtypes

import concourse.bass as bass
import concourse.bacc as bacc
import concourse.mybir as mybir
from concourse.tile import TileContext
from concourse.bass_utils import run_bass_kernel_spmd

F32 = mybir.dt.float32
F32R = mybir.dt.float32r
BF16 = mybir.dt.bfloat16
ATTN_BF16 = True
ADT = BF16 if ATTN_BF16 else F32R

B, S, D, H, DK = 4, 2048, 1024, 16, 64
NCORES = 8
NPAIR = 4               # head pairs per core
QC = 512                # q chunk (matmul moving free size)
NQC = S // QC           # 4
KC = 128                # k chunk (scores psum partition dim)
NKC = S // KC           # 16
SC = 512                # s chunk for projections / outproj
NSC = S // SC           # 4

_BUILT = {}


def _build_nc():
    nc = bacc.Bacc()

    xT_d = nc.declare_dram_parameter("xT", [D, S], BF16, isOutput=False)
    wq_d = nc.declare_dram_parameter("wqT", [D, 512], BF16, isOutput=False)
    wk_d = nc.declare_dram_parameter("wkT", [D, 512], BF16, isOutput=False)
    wv_d = nc.declare_dram_parameter("wvT", [D, 512], BF16, isOutput=False)
    wo_d = nc.declare_dram_parameter("woT", [512, D], BF16, isOutput=False)
    ctab_d = nc.declare_dram_parameter("ctab", [128, S], F32, isOutput=False)
    stab_d = nc.declare_dram_parameter("stab", [128, S], F32, isOutput=False)
    mk_d = nc.declare_dram_parameter("masks", [KC, 4, QC], ADT, isOutput=False)
    id_d = nc.declare_dram_parameter("ident", [128, 128], ADT, isOutput=False)
    idT_d = nc.declare_dram_parameter("identT", [128, 64], F32, isOutput=False)
    ones16_d = nc.declare_dram_parameter("ones16", [128, NKC], ADT, isOutput=False)
    out_d = nc.declare_dram_parameter("outP", [D, S], F32, isOutput=True)

    swapmask = [i ^ 1 for i in range(32)]

    with TileContext(nc) as tc, ExitStack() as ctx:
        ep = ctx.enter_context
        consts = ep(tc.tile_pool(name="consts", bufs=1))
        xpool = ep(tc.tile_pool(name="xt", bufs=8))
        wpool = ep(tc.tile_pool(name="w", bufs=36))
        wopool = ep(tc.tile_pool(name="wo", bufs=8))
        vpool = ep(tc.tile_pool(name="vraw", bufs=1))
        rupool = ep(tc.tile_pool(name="ru", bufs=2))
        rvpool = ep(tc.tile_pool(name="rv", bufs=2))
        qkpool = ep(tc.tile_pool(name="qk", bufs=4))
        v1pool = ep(tc.tile_pool(name="v1", bufs=3))
        ppool = ep(tc.tile_pool(name="pT", bufs=2))
        orawpool = ep(tc.tile_pool(name="oraw", bufs=1))
        sumpool = ep(tc.tile_pool(name="sums", bufs=2))
        recpool = ep(tc.tile_pool(name="rec", bufs=2))
        otpool = ep(tc.tile_pool(name="oT", bufs=4))
        obpool = ep(tc.tile_pool(name="ob", bufs=2))
        drpool = ep(tc.tile_pool(name="dr", bufs=4, space="DRAM"))
        psA = ep(tc.tile_pool(name="psA", bufs=2, space="PSUM"))
        psB = ep(tc.tile_pool(name="psB", bufs=2, space="PSUM"))
        psO = ep(tc.tile_pool(name="psO", bufs=2, space="PSUM"))

        ctab = consts.tile([128, S], F32)
        stab = consts.tile([128, S], F32)
        masks = consts.tile([KC, 4, QC], ADT)
        ident = consts.tile([128, 128], ADT)
        identT = consts.tile([128, 64], F32)
        nc.sync.dma_start(out=ctab, in_=ctab_d[:, :])
        nc.sync.dma_start(out=stab, in_=stab_d[:, :])
        nc.sync.dma_start(out=masks, in_=mk_d[:, :, :])
        nc.sync.dma_start(out=ident, in_=id_d[:, :])
        nc.sync.dma_start(out=identT, in_=idT_d[:, :])

        # resident x^T: 8 tiles [128, S] bf16 (4KB/partition each)
        xres = []
        for ic in range(8):
            xt = xpool.tile([128, S], BF16, tag="xt")
            nc.sync.dma_start(out=xt, in_=xT_d[ic * 128:(ic + 1) * 128, :])
            xres.append(xt)

        oTs = []
        for hp in range(NPAIR):
            # ---------------- projections for head pair hp ----------------
            wts = {}
            for pj, wsrc in enumerate((wq_d, wk_d, wv_d)):
                for ic in range(8):
                    w = wpool.tile([128, 128], BF16, tag="w")
                    nc.sync.dma_start(
                        out=w, in_=wsrc[ic * 128:(ic + 1) * 128,
                                        hp * 128:(hp + 1) * 128])
                    wts[(pj, ic)] = w

            qT = qkpool.tile([128, S], ADT, tag="qk")
            kT = qkpool.tile([128, S], ADT, tag="qk")
            vraw = vpool.tile([128, S], F32, tag="vraw")

            _sc = nc.named_scope(f"proj{hp}"); _sc.__enter__()
            for sc in range(NSC):
                ssl = slice(sc * SC, (sc + 1) * SC)
                for pj in range(3):
                    ps = psA.tile([128, SC], F32, tag="psA")
                    for ic in range(8):
                        nc.tensor.matmul(ps, wts[(pj, ic)], xres[ic][:, ssl],
                                         start=(ic == 0), stop=(ic == 7))
                    if pj < 2:  # Q or K: RoPE directly from PSUM
                        dst = qT if pj == 0 else kT
                        sh = rupool.tile([128, SC], F32, tag="ru")
                        nc.vector.stream_shuffle(out=sh, in_=ps, mask=swapmask)
                        t1 = rvpool.tile([128, SC], F32, tag="rv")
                        nc.vector.tensor_mul(out=t1, in0=ps, in1=ctab[:, ssl])
                        t2 = rupool.tile([128, SC], F32, tag="ru")
                        nc.vector.tensor_mul(out=t2, in0=sh, in1=stab[:, ssl])
                        nc.vector.tensor_add(out=dst[:, ssl], in0=t1, in1=t2)
                    else:       # V: drain to SBUF for PE transpose
                        nc.scalar.copy(out=vraw[:, ssl], in_=ps)

            _sc.__exit__(None, None, None)
            # ---------------- V transpose: [dk, s] -> [s, dk] + ones col ----
            _sc = nc.named_scope(f"vt{hp}"); _sc.__enter__()
            v1s = []
            for h in range(2):
                v1 = v1pool.tile([128, NKC, 65], ADT, tag="v1")
                for half in range(2):
                    pvt = psB.tile([128, 512], F32, tag="big")
                    for j in range(8):
                        kc = half * 8 + j
                        nc.tensor.transpose(
                            pvt[:, j * 64:(j + 1) * 64],
                            vraw[h * 64:(h + 1) * 64, kc * 128:(kc + 1) * 128],
                            identT[h * 64:(h + 1) * 64, 0:64])
                    nc.vector.tensor_copy(
                        out=v1[:, half * 8:(half + 1) * 8, 0:64],
                        in_=pvt.rearrange("p (kc d) -> p kc d", d=64))
                nc.sync.dma_start(out=v1[:, :, 64:65],
                                  in_=ones16_d[:, :].unsqueeze(2))
                v1s.append(v1)

            _sc.__exit__(None, None, None)
            # ---------------- attention per head ----------------
            _sc = nc.named_scope(f"attn{hp}"); _sc.__enter__()
            oT = otpool.tile([128, S], ADT, tag="oT")
            oraw = orawpool.tile([128, S], F32, tag="oraw")
            for h in range(2):
                hs = slice(h * 64, (h + 1) * 64)
                sums = sumpool.tile([128, QC], F32, tag="sums")
                nc.vector.memset(sums, 1.0)
                for qc in range(NQC):
                    nact = 4 * qc + 4
                    qsl = slice(qc * QC, (qc + 1) * QC)
                    pquads = []
                    for pr in range(nact // 2):
                        psq = psB.tile([128, 1024], F32, tag="big")
                        for sl in range(2):
                            kc = pr * 2 + sl
                            csl = slice(sl * QC, (sl + 1) * QC)
                            moff = kc - 4 * qc
                            partial = moff >= 0
                            nc.tensor.matmul(
                                psq[:, csl],
                                kT[hs, kc * KC:(kc + 1) * KC],
                                qT[hs, qsl],
                                start=True, stop=(not partial))
                            if partial:
                                nc.tensor.matmul(
                                    psq[:, csl], ident, masks[:, moff, :],
                                    start=False, stop=True)
                        pq = ppool.tile([128, 1024], ADT, tag="pT")
                        nc.scalar.activation(
                            out=pq, in_=psq,
                            func=mybir.ActivationFunctionType.Exp, scale=0.125)
                        pquads.append(pq)
                    pso = psO.tile([65, QC], F32, tag="psO")
                    for kc in range(nact):
                        pr, sl = divmod(kc, 2)
                        nc.tensor.matmul(
                            pso, v1s[h][:, kc, :],
                            pquads[pr][:, sl * QC:(sl + 1) * QC],
                            start=(kc == 0), stop=(kc == nact - 1))
                    nc.vector.tensor_copy(out=oraw[hs, qsl], in_=pso[0:64, :])
                    nc.vector.tensor_copy(out=sums[32 * qc:32 * qc + 1, :],
                                          in_=pso[64:65, :])
                # batched normalization for this head
                rec = recpool.tile([128, QC], F32, tag="rec")
                nc.vector.reciprocal(out=rec, in_=sums)
                drt = drpool.tile([NQC, QC], F32)
                for qc in range(NQC):
                    nc.sync.dma_start(out=drt[qc:qc + 1, :],
                                      in_=rec[32 * qc:32 * qc + 1, :])
                for qc in range(NQC):
                    qsl = slice(qc * QC, (qc + 1) * QC)
                    recB = recpool.tile([128, QC], F32, tag="recB")
                    nc.sync.dma_start(out=recB[hs, :],
                                      in_=drt[qc:qc + 1, :].to_broadcast((64, QC)))
                    nc.vector.tensor_mul(out=oT[hs, qsl], in0=oraw[hs, qsl],
                                         in1=recB[hs, :])
            _sc.__exit__(None, None, None)
            oTs.append(oT)

        # ---------------- output projection ----------------
        _sc = nc.named_scope("outproj"); _sc.__enter__()
        for oc in range(8):
            wos = []
            for hp in range(NPAIR):
                w = wopool.tile([128, 128], BF16, tag="wo")
                nc.sync.dma_start(
                    out=w, in_=wo_d[hp * 128:(hp + 1) * 128,
                                    oc * 128:(oc + 1) * 128])
                wos.append(w)
            for sc in range(NSC):
                ps = psA.tile([128, SC], F32, tag="psA")
                for hp in range(NPAIR):
                    nc.tensor.matmul(ps, wos[hp],
                                     oTs[hp][:, sc * SC:(sc + 1) * SC],
                                     start=(hp == 0), stop=(hp == NPAIR - 1))
                ob = obpool.tile([128, SC], F32, tag="ob")
                nc.vector.tensor_copy(out=ob, in_=ps)
                nc.sync.dma_start(
                    out=out_d[oc * 128:(oc + 1) * 128, sc * SC:(sc + 1) * SC],
                    in_=ob)
        _sc.__exit__(None, None, None)

    nc.compile()
    return nc


def get_nc():
    if "nc" not in _BUILT:
        _BUILT["nc"] = _build_nc()
    return _BUILT["nc"]


def _host_prep(x, Wq, Wk, Wv, Wo, token_positions):
    pos = np.asarray(token_positions).astype(np.float32)
    half = DK // 2
    inv_freq = 1.0 / (10000.0 ** (np.arange(half, dtype=np.float32) * 2.0 / DK))
    ang = pos[:, None] * inv_freq[None, :]          # [S, 32]
    cos = np.cos(ang).astype(np.float32)            # [S, 32]
    sin = np.sin(ang).astype(np.float32)
    p = np.arange(128)
    j = (p % 64) // 2
    sign = np.where(p % 2 == 0, -1.0, 1.0).astype(np.float32)
    ctab = np.ascontiguousarray(cos[:, j].T)                      # [128, S]
    stab = np.ascontiguousarray(sin[:, j].T * sign[:, None])      # [128, S]

    kk = np.arange(KC)[:, None]
    qq = np.arange(QC)[None, :]
    adt = ml_dtypes.bfloat16 if ATTN_BF16 else np.float32
    masks = np.stack([np.where(qq >= kk + 128 * m, 0.0, -1e9)
                      for m in range(4)], axis=1).astype(adt)  # [KC,4,QC]
    ident = np.eye(128, dtype=adt)
    identT = np.vstack([np.eye(64, dtype=np.float32)] * 2)
    ones16 = np.ones((128, NKC), dtype=adt)

    bf = ml_dtypes.bfloat16
    in_maps = []
    for c in range(NCORES):
        b, hf = divmod(c, 2)
        m = {}
        m["xT"] = np.ascontiguousarray(x[b].T).astype(bf)  # [D, S]
        m["wqT"] = np.ascontiguousarray(Wq[hf * 512:(hf + 1) * 512, :].T).astype(bf)
        m["wkT"] = np.ascontiguousarray(Wk[hf * 512:(hf + 1) * 512, :].T).astype(bf)
        m["wvT"] = np.ascontiguousarray(Wv[hf * 512:(hf + 1) * 512, :].T).astype(bf)
        m["woT"] = np.ascontiguousarray(Wo[:, hf * 512:(hf + 1) * 512].T).astype(bf)
        m["ctab"] = ctab
        m["stab"] = stab
        m["masks"] = masks
        m["ident"] = ident
        m["identT"] = identT
        m["ones16"] = ones16
        in_maps.append(m)
    return in_maps


def run(inputs, trace=False, **kw):
    in_maps = _host_prep(**{k: np.asarray(v) for k, v in inputs.items()})
    nc = get_nc()
    res = run_bass_kernel_spmd(nc, in_maps, list(range(NCORES)), trace=trace, **kw)
    outs = [res.results[c]["outP"] for c in range(NCORES)]
    out = np.stack([(outs[2 * b] + outs[2 * b + 1]).T for b in range(B)])
    return out.astype(np.float32), res


def kernel(**inputs):
    out, _ = run(inputs, trace=False)
    return out

